# revision 40
# baseline (speedup 1.0000x reference)
"""Multi-head self-attention (B=8, S=2048, H=256, NH=8, HD=32) on 8 TRN2 cores.

Strategy: data-parallel over batch — each core computes full MHA for one
batch element; no collectives.

Per-core dataflow (all matmuls bf16 in / fp32 PSUM accum):
  - host ships x^T (features on partitions) so no on-device transpose
  - qkT:  q^T,k^T [feat, s] = w_qkv^T @ x — feature-major so each head's
    32 q/k features land on one 32-partition strip; bias folded into the
    eviction (split between ScalarE and VectorE, both idle here)
  - the attention loop runs over 256 blocks: (q-block 256) x (key-tile
    128) x (head-group of 4).  Per block, ONE [128, 4*256] scores PSUM
    lane is filled by 4 row-tiled PE matmuls (tile_position=(32i,0),
    K=32), consumed by ONE 1024-column exp op.  THREE lanes rotate over
    blocks so consecutive blocks' exps never wait on each other's
    buffer WAR — both exp engines always have a queued op.
  - softmax exp is the kernel bottleneck (NH*S*S = 33.5M elements/core,
    and exp natively runs only on ScalarE at 1 col/cycle).  The exp work
    is therefore SPLIT between two engines:
      * ScalarE share: ACTIVATE(Exp, scale=1/sqrt(HD)) from PSUM
      * VectorE share: one TENSOR_SCALAR computing the Schraudolph bit
        trick — i16 = trunc(score * (128*log2(e)/sqrt(HD)) + (127*128-C))
        written through an int16 bitcast of the bf16 eT tile.  The int16
        bit pattern IS bf16(exp(score/sqrt(HD))) up to ~2% sawtooth error
        which largely cancels under softmax renormalization (validated:
        global rel err 0.008 vs 0.005 for exact exp).
    A 16-block pattern interleaves the engines ~9:7 to balance
    ScalarE@1.2GHz against VectorE@0.96GHz + its other work.
  - softmax max-subtraction is skipped (scores are O(1), fp32 PSUM)
  - ctx^T accumulated over key tiles with 4x column-tiled PE
    (tile_position=(0,32h)): all 8 heads of BOTH groups land fully
    packed in ONE [128,512] PSUM bank (g0 cols 0:256, g1 cols 256:512);
    rowsums likewise accumulate as M=1 column-tiled matmuls against a
    ones column into a second shared bank (row h*32, same col split)
  - dep-free filler matmuls (zeros column accumulating +0 into the live
    rowsum rows) keep the PE instruction stream gapless so the HAM
    activity monitor never re-throttles the PE clock 2.4 -> 1.2 GHz
    (without them a MID window fires ~20µs in and the whole attention
    loop runs at half clock); the tail of the v-projection is deferred
    into the first q-block for the same reason
  - normalization off the critical path: ctx/rowsum banks evicted by
    single full-tile engine copies into [128,512] fp32 staging (frees
    the accumulators), rowsum rows shipped via DRAM into one [128,16]
    tile, one VectorE reciprocal, scattered back and partition-broadcast
    via DRAM, then 2 [128,256] multiplies write the normalized bf16
    ctxT.  All pieces are emitted a few blocks into the NEXT q-block so
    the DMA round-trips overlap exp work instead of head-of-line
    blocking the exp streams.
  - out = ctxT^T @ w_out + b_out: ctxT is fully packed (2 tiles of
    4 heads x 32 rows), w_out needs no permutation, bias comes from a
    K=1 ones-row matmul.
"""
import math

import numpy as np
import ml_dtypes

import bass_rust
import concourse.bass as bass
import concourse.mybir as mybir
import concourse.tile as tile
from concourse.vector_clock import ScopedClock
from concourse.bass_utils import run_bass_kernel_spmd

BF16 = mybir.dt.bfloat16
F32 = mybir.dt.float32
NPBF16 = ml_dtypes.bfloat16

B, S, H = 8, 2048, 256
NH, HD = 8, 32
SCALE = 1.0 / float(np.sqrt(HD))
N_CORES = 8
QB = 512  # q-block size
NQB = S // QB  # 4 q-blocks (x2 head-group passes = 8 passes)

# Schraudolph-exp constants for the VectorE share: bf16 bit pattern of
# exp(SCALE*x) ~= trunc(x * A16 + B16) interpreted as int16.
A16 = SCALE * 128.0 / math.log(2.0)
B16 = 127.0 * 128.0 - 6.0

# Which of each 16 consecutive (qb,kt,g) exp blocks go to VectorE
# (7 of 16; the rest go to ScalarE).  Evenly interleaved.
DVE_SLOTS = frozenset(i for i in range(16) if (i * 7) % 16 < 7)

FILLERS = True  # warm-keeper filler matmuls (see phase 3)

# Set by a test harness to collect HW timing: {"trace": bool, "trace_cores": [...]}
TRACE_OPTS = {}
LAST_RESULT = None

def _legalize_sync_waits(nc):
    """The walrus build here rejects >1 sync wait per instruction, but Tile
    freely emits 2-3 (and the exit drain up to ~27).  Move excess waits onto
    same-engine NoOp carriers inserted immediately before the offending
    instruction — identical semantics (the engine blocks on each wait in
    program order)."""
    n = 0
    for f in nc.m.functions:
        for bb in f.blocks:
            insts = bb.instructions  # live list
            i = 0
            while i < len(insts):
                inst = insts[i]
                si = inst.sync_info
                if si is not None and len(si.on_wait) > 1:
                    waits = list(si.on_wait)
                    carriers = []
                    for w in waits[:-1]:
                        carriers.append(
                            mybir.InstNoOp(
                                name=f"{inst.name}-w{n}",
                                sync_info=mybir.SyncInfo(on_wait=[w], on_update=[]),
                                bass_nofuse=True,
                                engine=inst.engine,
                            )
                        )
                        n += 1
                    inst.sync_info = bass_rust.SyncInfo(
                        on_wait=waits[-1:], on_update=list(si.on_update)
                    )
                    insts[i:i] = carriers
                    i += len(carriers)
                i += 1
    return n


def _build_nc(legalize=True):
    nc = bass.Bass()
    xt = nc.dram_tensor("xt", [128, 2 * S], BF16, kind="ExternalInput")
    wqk = nc.dram_tensor("wqk", [128, 2 * 512], BF16, kind="ExternalInput")
    bv = nc.dram_tensor("bv", [1, 256], BF16, kind="ExternalInput")
    wv = nc.dram_tensor("wv", [128, 2 * 256], BF16, kind="ExternalInput")
    wo = nc.dram_tensor("wo", [128, 2 * 256], BF16, kind="ExternalInput")
    bo = nc.dram_tensor("bo", [1, 256], BF16, kind="ExternalInput")
    bqkc = nc.dram_tensor("bqkc", [128, 4], F32, kind="ExternalInput")
    ones = nc.dram_tensor("ones", [1, 512], BF16, kind="ExternalInput")
    out = nc.dram_tensor("out", [S, H], F32, kind="ExternalOutput")
    # scratch for the rowsum gather / reciprocal-broadcast DMA roundtrips
    # (SBUF APs cannot have a zero partition step, DRAM APs can)
    rscr = nc.dram_tensor("rscr", [32, 512], F32)
    rscr2 = nc.dram_tensor("rscr2", [32, 512], F32)

    EXP = mybir.ActivationFunctionType.Exp
    IDN = mybir.ActivationFunctionType.Identity
    MUL = mybir.AluOpType.mult
    ADD = mybir.AluOpType.add

    with tile.TileContext(nc) as tc:
        with (
            tc.tile_pool(name="const", bufs=1) as const,
            tc.tile_pool(name="etp", bufs=6) as etp,
            tc.tile_pool(name="nrm", bufs=2) as nrm,
        ):
            xt_sb = const.tile([128, 2 * S], BF16, tag="xt")
            nc.sync.dma_start(out=xt_sb, in_=xt[:, :])
            wqk_sb = const.tile([128, 2 * 512], BF16, tag="wqk")
            nc.sync.dma_start(out=wqk_sb, in_=wqk[:, :])
            wv_sb = const.tile([128, 2 * 256], BF16, tag="wv")
            nc.sync.dma_start(out=wv_sb, in_=wv[:, :])
            wo_sb = const.tile([128, 2 * 256], BF16, tag="wo")
            nc.sync.dma_start(out=wo_sb, in_=wo[:, :])
            bv_sb = const.tile([1, 256], BF16, tag="bv")
            nc.sync.dma_start(out=bv_sb, in_=bv[:, :])
            bo_sb = const.tile([1, 256], BF16, tag="bo")
            nc.sync.dma_start(out=bo_sb, in_=bo[:, :])
            ones1_sb = const.tile([1, 128], BF16, tag="ones1")
            nc.sync.dma_start(out=ones1_sb, in_=ones[0:1, 0:128])
            bqkc_sb = const.tile([128, 4], F32, tag="bqkc")
            nc.sync.dma_start(out=bqkc_sb, in_=bqkc[:, :])
            # ones block for the rowsum matmuls (K=128, M=32: 32 identical
            # rowsum rows cost the same PE time as 1 and leave no stale
            # bytes in the shared PSUM bank) and a zeros column for the
            # harmless warm-keeper fillers
            onec_sb = const.tile([128, 32], BF16, tag="onec")
            nc.vector.memset(onec_sb, 1.0)
            zeroc_sb = const.tile([128, 1], BF16, tag="zeroc")
            nc.vector.memset(zeroc_sb, 0.0)

            qT_sb = const.tile([128, 2 * S], BF16, tag="qT")
            kT_sb = const.tile([128, 2 * S], BF16, tag="kT")
            v_sb = const.tile([128, 16 * 256], BF16, tag="v")
            # fully-packed normalized ctx^T: tile g holds heads 4g..4g+3,
            # rows h*32..h*32+32 = head (4g+h) features, cols = q positions
            ctxT_sb = [
                const.tile([128, S], BF16, tag=f"ctxT{g}", name=f"ctxT{g}")
                for g in range(2)
            ]

            # ---- phase 0: HAM warmup — ~6µs of dep-free back-to-back
            # matmuls so the PE clock gate opens (1.2 -> 2.4 GHz) before the
            # real work; garbage values land in a scratch PSUM bank that is
            # never read.  A dummy exp on ScalarE pulls the ~2.7µs ACT
            # table load off the critical path too. ----
            with tc.tile_pool(name="pp", bufs=4, space="PSUM") as pp:
                warm_sb = const.tile([128, 512], BF16, tag="warm")
                nc.vector.memset(warm_sb, 0.0)
                dume_sb = const.tile([1, 16], BF16, tag="dume")
                nc.scalar.activation(
                    out=dume_sb, in_=warm_sb[0:1, 0:16], func=EXP, scale=SCALE
                )
                warm_ps = pp.tile([128, 512], F32, tag="pp")
                for _ in range(12):
                    nc.tensor.matmul(
                        out=warm_ps, lhsT=warm_sb[:, 0:128], rhs=warm_sb[:, :],
                        start=True, stop=True,
                    )

                # ---- phase 1: qT/kT [feature, s] = w_qkv^T @ x; bias folded
                #      into the eviction (per-partition, features-major),
                #      evictions alternating ScalarE/VectorE ----
                for t in range(4):  # feature tiles: q0,q1,k0,k1
                    for nb in range(4):  # s blocks of 512
                        ps = pp.tile([128, 512], F32, tag="pp")
                        for ks in range(2):
                            nc.tensor.matmul(
                                out=ps,
                                lhsT=wqk_sb[:, ks * 512 + t * 128 : ks * 512 + t * 128 + 128],
                                rhs=xt_sb[:, ks * S + nb * 512 : ks * S + nb * 512 + 512],
                                start=(ks == 0), stop=(ks == 1),
                            )
                        dst = (qT_sb if t < 2 else kT_sb)[
                            :, (t % 2) * S + nb * 512 : (t % 2) * S + nb * 512 + 512
                        ]
                        if (t * 4 + nb) % 2 == 0:
                            nc.scalar.activation(
                                out=dst, in_=ps, func=IDN,
                                bias=bqkc_sb[:, t : t + 1], scale=1.0,
                            )
                        else:
                            nc.vector.tensor_scalar_add(
                                out=dst, in0=ps, scalar1=bqkc_sb[:, t : t + 1]
                            )

                # ---- phase 2: v (natural layout, 32-wide head slots),
                #      evictions alternating ScalarE/VectorE ----
                for st in range(16):
                    ps = pp.tile([128, 256], F32, tag="ppv")
                    for ks in range(2):
                        nc.tensor.matmul(
                            out=ps,
                            lhsT=xt_sb[:, ks * S + st * 128 : ks * S + st * 128 + 128],
                            rhs=wv_sb[:, ks * 256 : ks * 256 + 256],
                            start=(ks == 0), stop=False,
                        )
                    nc.tensor.matmul(
                        out=ps,
                        lhsT=ones1_sb[0:1, 0:128],
                        rhs=bv_sb[0:1, 0:256],
                        start=False, stop=True,
                    )
                    dst = v_sb[:, st * 256 : st * 256 + 256]
                    if st % 2 == 0:
                        nc.scalar.copy(out=dst, in_=ps)
                    else:
                        nc.vector.tensor_copy(out=dst, in_=ps)

            # ---- phase 3: attention (pp pool closed).  Runs as 8
            #      sequential passes (q-block 512) x (head-group of 4);
            #      per pass 16 key-tiles x 2 half-blocks of 2 heads.
            #      PSUM: 3 score lanes [128,1024] (6 banks; each of the
            #      two concurrent row-tiled score matmuls in a lane owns
            #      a FULL bank — two row-tiled matmuls draining into one
            #      single-port bank SRAM is a fatal PSUM collision) +
            #      1 ctx bank + 1 rowsum bank = 8. ----
            hidx = 0  # global exp-half-block counter for the engine split

            if True:
                def evict_pass(p, ctx_ps, rs_ps):
                    # engine copies out of PSUM (frees the accumulator
                    # banks), then ship the 4 rowsum rows to DRAM and
                    # gather them back as one [64, 32] tile
                    stg = nrm.tile([128, 512], F32, tag="stg", name=f"stg_{p}")
                    nc.scalar.copy(out=stg, in_=ctx_ps)
                    rss = nrm.tile([128, 512], F32, tag="rss", name=f"rss_{p}")
                    nc.vector.tensor_copy(out=rss, in_=rs_ps)
                    for hh in range(4):
                        nc.sync.dma_start(
                            out=rscr[p * 4 + hh : p * 4 + hh + 1, :],
                            in_=rss[hh * 32 : hh * 32 + 1, :],
                        )
                    rsg = nrm.tile([64, 32], F32, tag="rsg", name=f"rsg_{p}")
                    nc.sync.dma_start(
                        out=rsg,
                        in_=rscr[p * 4 : p * 4 + 4, :].rearrange(
                            "r (c k) -> (r c) k", k=32
                        ),
                    )
                    return stg, rsg

                def norm_a(p, stg, rsg):
                    # reciprocal + scatter + partition-broadcast roundtrip
                    rsgi = nrm.tile([64, 32], F32, tag="rsgi", name=f"rsgi_{p}")
                    nc.vector.reciprocal(out=rsgi, in_=rsg)
                    nc.sync.dma_start(
                        out=rscr2[p * 4 : p * 4 + 4, :].rearrange(
                            "r (c k) -> (r c) k", k=32
                        ),
                        in_=rsgi,
                    )
                    rcb = nrm.tile([128, 512], F32, tag="rcb", name=f"rcb_{p}")
                    for hh in range(4):
                        nc.sync.dma_start(
                            out=rcb[hh * 32 : hh * 32 + 32, :],
                            in_=rscr2[p * 4 + hh : p * 4 + hh + 1, :].to_broadcast((32, 512)),
                        )
                    return rcb

                def norm_b(p, stg, rcb):
                    qb, g = p // 2, p % 2
                    nc.vector.tensor_mul(
                        out=ctxT_sb[g][:, qb * 512 : qb * 512 + 512],
                        in0=stg, in1=rcb,
                    )

                with (
                    tc.tile_pool(name="scp", bufs=3, space="PSUM") as scp,
                    tc.tile_pool(name="cxp", bufs=1, space="PSUM") as cxp,
                ):
                    pending_norm = None
                    prev_rs = None
                    for p in range(8):  # pass = (q-block, head-group)
                        qb, g = p // 2, p % 2
                        ctx_ps = cxp.tile([128, 512], F32, tag="ctx", name=f"ctx_{p}")
                        rs_ps = cxp.tile([128, 512], F32, tag="rs", name=f"rs_{p}")
                        rs_started = False

                        def emit_ctx(kt, eTs):
                            # ctx + rowsum accumulation for key-tile kt —
                            # one block late so these PE matmuls overlap
                            # the next exps.  4 column-tiled matmuls per
                            # wave (distinct partition groups -> distinct
                            # PSUM write ports, safe to share the bank).
                            for hh in range(4):
                                nc.tensor.matmul(
                                    out=ctx_ps[hh * 32 : hh * 32 + 32, :],
                                    lhsT=v_sb[:, kt * 256 + (g * 4 + hh) * 32 : kt * 256 + (g * 4 + hh) * 32 + 32],
                                    rhs=eTs[hh // 2][:, (hh % 2) * 512 : (hh % 2) * 512 + 512],
                                    start=(kt == 0), stop=(kt == 15),
                                    tile_position=(0, hh * 32), skip_group_check=True,
                                )
                            for hh in range(4):
                                nc.tensor.matmul(
                                    out=rs_ps[hh * 32 : hh * 32 + 32, :],
                                    lhsT=onec_sb[:, 0:32],
                                    rhs=eTs[hh // 2][:, (hh % 2) * 512 : (hh % 2) * 512 + 512],
                                    start=(kt == 0), stop=(kt == 15),
                                    tile_position=(0, hh * 32), skip_group_check=True,
                                )

                        def filler(rs_tile, n=256):
                            # accumulate +0 into a live rowsum row: pure PE
                            # busyness, numerically a no-op
                            nc.tensor.matmul(
                                out=rs_tile[0:1, 0:n],
                                lhsT=zeroc_sb[:, 0:1],
                                rhs=warm_sb[:, 0:n],
                                start=False, stop=False, skip_group_check=True,
                            )

                        pending = None  # (kt, [eT_half0, eT_half1])
                        cur_eTs = None
                        for kt in range(16):
                            # previous pass's normalize, emitted a couple
                            # of blocks into this one so the DMA roundtrip
                            # overlaps exp work instead of head-of-line
                            # blocking the exp streams
                            if pending_norm is not None and kt == 2:
                                pp_, pstg, prsg = pending_norm
                                pending_norm = (pp_, pstg, norm_a(pp_, pstg, prsg))
                            if pending_norm is not None and kt == 4:
                                pp_, pstg, prcb = pending_norm
                                norm_b(pp_, pstg, prcb)
                                pending_norm = None
                            cur_eTs = []
                            for half in range(2):
                                eT = etp.tile([128, 1024], BF16, tag="eT")
                                sc = scp.tile([128, 1024], F32, tag="sc",
                                              name=f"sc_{p}_{kt}_{half}")
                                for j in range(2):
                                    hh = 2 * half + j
                                    nc.tensor.matmul(
                                        out=sc[:, j * 512 : j * 512 + 512],
                                        lhsT=kT_sb[32 * hh : 32 * hh + 32,
                                                   g * S + kt * 128 : g * S + kt * 128 + 128],
                                        rhs=qT_sb[32 * hh : 32 * hh + 32,
                                                  g * S + qb * 512 : g * S + qb * 512 + 512],
                                        start=True, stop=True,
                                        tile_position=(32 * hh, 0),
                                    )
                                if (hidx % 16) in DVE_SLOTS:
                                    nc.vector.tensor_scalar(
                                        out=eT.bitcast(mybir.dt.int16),
                                        in0=sc, scalar1=A16, scalar2=B16,
                                        op0=MUL, op1=ADD,
                                    )
                                else:
                                    nc.scalar.activation(
                                        out=eT, in_=sc, func=EXP, scale=SCALE,
                                    )
                                hidx += 1
                                cur_eTs.append(eT)
                                # ctx/rs wave for the previous key-tile sits
                                # between this block's two halves
                                if half == 0 and pending is not None:
                                    emit_ctx(*pending)
                                    if pending[0] == 0:
                                        rs_started = True
                                    pending = None
                                # zero-add fillers keep the PE stream
                                # gapless so HAM never re-throttles
                                if FILLERS:
                                    if rs_started:
                                        filler(rs_ps)
                                    elif prev_rs is not None:
                                        filler(prev_rs)
                            pending = (kt, cur_eTs)
                        emit_ctx(*pending)
                        # eviction emitted before the next pass re-allocates
                        # these PSUM buffers
                        pending_norm = (p,) + evict_pass(p, ctx_ps, rs_ps)
                        prev_rs = rs_ps

                    # tail: normalize the last pass
                    pp_, pstg, prsg = pending_norm
                    rcb = norm_a(pp_, pstg, prsg)
                    norm_b(pp_, pstg, rcb)

            # ---- phase 4: out = ctxT^T @ w_out + b_out (K=1 ones-row
            #      matmul adds the bias) ----
            with (
                tc.tile_pool(name="op", bufs=4, space="PSUM") as op,
                tc.tile_pool(name="ev", bufs=4) as ev,
            ):
                for st in range(16):
                    ps = op.tile([128, 256], F32, tag="op")
                    nc.tensor.matmul(
                        out=ps, lhsT=ones1_sb[0:1, 0:128], rhs=bo_sb[0:1, :],
                        start=True, stop=False,
                    )
                    for g in range(2):
                        nc.tensor.matmul(
                            out=ps,
                            lhsT=ctxT_sb[g][:, st * 128 : st * 128 + 128],
                            rhs=wo_sb[:, g * 256 : g * 256 + 256],
                            start=False, stop=(g == 1),
                        )
                    ot = ev.tile([128, 256], F32, tag="ot")
                    if st % 2 == 0:
                        nc.scalar.copy(out=ot, in_=ps)
                    else:
                        nc.vector.tensor_copy(out=ot, in_=ps)
                    nc.sync.dma_start(
                        out=out[st * 128 : st * 128 + 128, :], in_=ot
                    )
    if legalize:
        _legalize_sync_waits(nc)
    return nc


_NC_CACHE = None


def _get_nc():
    global _NC_CACHE
    if _NC_CACHE is None:
        _NC_CACHE = _build_nc()
    return _NC_CACHE


def _ks_layout(a, nk, cols):
    """[nk*128, cols] -> [128, nk*cols] with [p, k*cols+c] = a[k*128+p, c]."""
    return np.ascontiguousarray(
        a.reshape(nk, 128, cols).transpose(1, 0, 2).reshape(128, nk * cols)
    )


def _prep_in_maps(x, w_qkv, b_qkv, w_out, b_out):
    x = np.asarray(x, dtype=np.float32)
    w_qkv = np.asarray(w_qkv, dtype=np.float32)
    b_qkv = np.asarray(b_qkv, dtype=np.float32)
    w_out = np.asarray(w_out, dtype=np.float32)
    b_out = np.asarray(b_out, dtype=np.float32)

    # shared (per-core identical) weight layouts
    wqk_l = _ks_layout(w_qkv[:, : 2 * H], 2, 512).astype(NPBF16)
    # v weights in natural head order (32-wide slots)
    wv_l = _ks_layout(w_qkv[:, 2 * H :], 2, 256).astype(NPBF16)
    # out projection: ctxT is packed [(head h)*32 + d] so w_out needs no
    # permutation, only the K-split layout
    wo_l = _ks_layout(w_out, 2, 256).astype(NPBF16)

    shared = {
        "wqk": wqk_l,
        "wv": wv_l,
        "bv": b_qkv[2 * H :].reshape(1, H).astype(NPBF16),
        "wo": wo_l,
        "bo": b_out.reshape(1, H).astype(NPBF16),
        "bqkc": np.ascontiguousarray(
            b_qkv[: 2 * H].astype(np.float32).reshape(4, 128).T
        ),
        "ones": np.ones((1, 512), NPBF16),
    }
    in_maps = []
    for b in range(B):
        xt = _ks_layout(np.ascontiguousarray(x[b].T), 2, S).astype(NPBF16)
        in_maps.append({"xt": xt, **shared})
    return in_maps


def kernel(x, w_qkv, b_qkv, w_out, b_out):
    in_maps = _prep_in_maps(x, w_qkv, b_qkv, w_out, b_out)
    nc = _get_nc()
    res = run_bass_kernel_spmd(nc, in_maps, list(range(N_CORES)), **TRACE_OPTS)
    global LAST_RESULT
    LAST_RESULT = res
    return np.stack([res.results[b]["out"] for b in range(B)], axis=0)


# revision 41
# speedup vs baseline: 1.4775x; 1.4775x over previous
"""Multi-head self-attention (B=8, S=2048, H=256, NH=8, HD=32) on 8 TRN2 cores.

Strategy: data-parallel over batch — each core computes full MHA for one
batch element; no collectives.

Per-core dataflow (all matmuls bf16 in / fp32 PSUM accum):
  - host ships x^T (features on partitions) so no on-device transpose
  - qkT:  q^T,k^T [feat, s] = w_qkv^T @ x — feature-major so each head's
    32 q/k features land on one 32-partition strip; bias folded into the
    eviction (split between ScalarE and VectorE, both idle here)
  - the attention loop runs over 256 blocks: (q-block 256) x (key-tile
    128) x (head-group of 4).  Per block, ONE [128, 4*256] scores PSUM
    lane is filled by 4 row-tiled PE matmuls (tile_position=(32i,0),
    K=32), consumed by ONE 1024-column exp op.  THREE lanes rotate over
    blocks so consecutive blocks' exps never wait on each other's
    buffer WAR — both exp engines always have a queued op.
  - softmax exp is the kernel bottleneck (NH*S*S = 33.5M elements/core,
    and exp natively runs only on ScalarE at 1 col/cycle).  The exp work
    is therefore SPLIT between two engines:
      * ScalarE share: ACTIVATE(Exp, scale=1/sqrt(HD)) from PSUM
      * VectorE share: one TENSOR_SCALAR computing the Schraudolph bit
        trick — i16 = trunc(score * (128*log2(e)/sqrt(HD)) + (127*128-C))
        written through an int16 bitcast of the bf16 eT tile.  The int16
        bit pattern IS bf16(exp(score/sqrt(HD))) up to ~2% sawtooth error
        which largely cancels under softmax renormalization (validated:
        global rel err 0.008 vs 0.005 for exact exp).
    A 16-block pattern interleaves the engines ~9:7 to balance
    ScalarE@1.2GHz against VectorE@0.96GHz + its other work.
  - softmax max-subtraction is skipped (scores are O(1), fp32 PSUM)
  - ctx^T accumulated over key tiles with 4x column-tiled PE
    (tile_position=(0,32h)): all 8 heads of BOTH groups land fully
    packed in ONE [128,512] PSUM bank (g0 cols 0:256, g1 cols 256:512);
    rowsums likewise accumulate as M=1 column-tiled matmuls against a
    ones column into a second shared bank (row h*32, same col split)
  - dep-free filler matmuls (zeros column accumulating +0 into the live
    rowsum rows) keep the PE instruction stream gapless so the HAM
    activity monitor never re-throttles the PE clock 2.4 -> 1.2 GHz
    (without them a MID window fires ~20µs in and the whole attention
    loop runs at half clock); the tail of the v-projection is deferred
    into the first q-block for the same reason
  - normalization off the critical path: ctx/rowsum banks evicted by
    single full-tile engine copies into [128,512] fp32 staging (frees
    the accumulators), rowsum rows shipped via DRAM into one [128,16]
    tile, one VectorE reciprocal, scattered back and partition-broadcast
    via DRAM, then 2 [128,256] multiplies write the normalized bf16
    ctxT.  All pieces are emitted a few blocks into the NEXT q-block so
    the DMA round-trips overlap exp work instead of head-of-line
    blocking the exp streams.
  - out = ctxT^T @ w_out + b_out: ctxT is fully packed (2 tiles of
    4 heads x 32 rows), w_out needs no permutation, bias comes from a
    K=1 ones-row matmul.
"""
import math

import numpy as np
import ml_dtypes

import bass_rust
import concourse.bass as bass
import concourse.mybir as mybir
import concourse.tile as tile
from concourse.vector_clock import ScopedClock
from concourse.bass_utils import run_bass_kernel_spmd

BF16 = mybir.dt.bfloat16
F32 = mybir.dt.float32
NPBF16 = ml_dtypes.bfloat16

B, S, H = 8, 2048, 256
NH, HD = 8, 32
SCALE = 1.0 / float(np.sqrt(HD))
N_CORES = 8
QB = 512  # q-block size
NQB = S // QB  # 4 q-blocks (x2 head-group passes = 8 passes)

# Schraudolph-exp constants for the VectorE share: bf16 bit pattern of
# exp(SCALE*x) ~= trunc(x * A16 + B16) interpreted as int16.
A16 = SCALE * 128.0 / math.log(2.0)
B16 = 127.0 * 128.0 - 6.0

# Which of each 16 consecutive (qb,kt,g) exp blocks go to VectorE
# (7 of 16; the rest go to ScalarE).  Evenly interleaved.
DVE_SLOTS = frozenset(i for i in range(16) if (i * 7) % 16 < 7)

FILLERS = True  # warm-keeper filler matmuls (see phase 3)

# Set by a test harness to collect HW timing: {"trace": bool, "trace_cores": [...]}
TRACE_OPTS = {}
LAST_RESULT = None

def _legalize_sync_waits(nc):
    """The walrus build here rejects >1 sync wait per instruction, but Tile
    freely emits 2-3 (and the exit drain up to ~27).  Move excess waits onto
    same-engine NoOp carriers inserted immediately before the offending
    instruction — identical semantics (the engine blocks on each wait in
    program order)."""
    n = 0
    for f in nc.m.functions:
        for bb in f.blocks:
            insts = bb.instructions  # live list
            i = 0
            while i < len(insts):
                inst = insts[i]
                si = inst.sync_info
                if si is not None and len(si.on_wait) > 1:
                    waits = list(si.on_wait)
                    carriers = []
                    for w in waits[:-1]:
                        carriers.append(
                            mybir.InstNoOp(
                                name=f"{inst.name}-w{n}",
                                sync_info=mybir.SyncInfo(on_wait=[w], on_update=[]),
                                bass_nofuse=True,
                                engine=inst.engine,
                            )
                        )
                        n += 1
                    inst.sync_info = bass_rust.SyncInfo(
                        on_wait=waits[-1:], on_update=list(si.on_update)
                    )
                    insts[i:i] = carriers
                    i += len(carriers)
                i += 1
    return n


def _build_nc(legalize=True):
    nc = bass.Bass()
    xt = nc.dram_tensor("xt", [128, 2 * S], BF16, kind="ExternalInput")
    wqk = nc.dram_tensor("wqk", [128, 2 * 512], BF16, kind="ExternalInput")
    bv = nc.dram_tensor("bv", [1, 256], BF16, kind="ExternalInput")
    wv = nc.dram_tensor("wv", [128, 2 * 256], BF16, kind="ExternalInput")
    wo = nc.dram_tensor("wo", [128, 2 * 256], BF16, kind="ExternalInput")
    bo = nc.dram_tensor("bo", [1, 256], BF16, kind="ExternalInput")
    bqkc = nc.dram_tensor("bqkc", [128, 4], F32, kind="ExternalInput")
    ones = nc.dram_tensor("ones", [1, 512], BF16, kind="ExternalInput")
    out = nc.dram_tensor("out", [S, H], F32, kind="ExternalOutput")
    # scratch for the rowsum gather / reciprocal-broadcast DMA roundtrips
    # (SBUF APs cannot have a zero partition step, DRAM APs can)
    rscr = nc.dram_tensor("rscr", [32, 512], F32)
    rscr2 = nc.dram_tensor("rscr2", [32, 512], F32)

    EXP = mybir.ActivationFunctionType.Exp
    IDN = mybir.ActivationFunctionType.Identity
    MUL = mybir.AluOpType.mult
    ADD = mybir.AluOpType.add

    with tile.TileContext(nc) as tc:
        with (
            tc.tile_pool(name="const", bufs=1) as const,
            tc.tile_pool(name="etp", bufs=6) as etp,
            tc.tile_pool(name="nrm", bufs=2) as nrm,
        ):
            xt_sb = const.tile([128, 2 * S], BF16, tag="xt")
            nc.sync.dma_start(out=xt_sb, in_=xt[:, :])
            wqk_sb = const.tile([128, 2 * 512], BF16, tag="wqk")
            nc.sync.dma_start(out=wqk_sb, in_=wqk[:, :])
            wv_sb = const.tile([128, 2 * 256], BF16, tag="wv")
            nc.sync.dma_start(out=wv_sb, in_=wv[:, :])
            wo_sb = const.tile([128, 2 * 256], BF16, tag="wo")
            nc.sync.dma_start(out=wo_sb, in_=wo[:, :])
            bv_sb = const.tile([1, 256], BF16, tag="bv")
            nc.sync.dma_start(out=bv_sb, in_=bv[:, :])
            bo_sb = const.tile([1, 256], BF16, tag="bo")
            nc.sync.dma_start(out=bo_sb, in_=bo[:, :])
            ones1_sb = const.tile([1, 128], BF16, tag="ones1")
            nc.sync.dma_start(out=ones1_sb, in_=ones[0:1, 0:128])
            bqkc_sb = const.tile([128, 4], F32, tag="bqkc")
            nc.sync.dma_start(out=bqkc_sb, in_=bqkc[:, :])
            # ones block for the rowsum matmuls (K=128, M=32: 32 identical
            # rowsum rows cost the same PE time as 1 and leave no stale
            # bytes in the shared PSUM bank) and a zeros column for the
            # harmless warm-keeper fillers
            onec_sb = const.tile([128, 32], BF16, tag="onec")
            nc.vector.memset(onec_sb, 1.0)
            zeroc_sb = const.tile([128, 1], BF16, tag="zeroc")
            nc.vector.memset(zeroc_sb, 0.0)

            qT_sb = const.tile([128, 2 * S], BF16, tag="qT")
            kT_sb = const.tile([128, 2 * S], BF16, tag="kT")
            v_sb = const.tile([128, 16 * 256], BF16, tag="v")
            # fully-packed normalized ctx^T: tile g holds heads 4g..4g+3,
            # rows h*32..h*32+32 = head (4g+h) features, cols = q positions
            ctxT_sb = [
                const.tile([128, S], BF16, tag=f"ctxT{g}", name=f"ctxT{g}")
                for g in range(2)
            ]

            # ---- phase 0: HAM warmup — ~6µs of dep-free back-to-back
            # matmuls so the PE clock gate opens (1.2 -> 2.4 GHz) before the
            # real work; garbage values land in a scratch PSUM bank that is
            # never read.  A dummy exp on ScalarE pulls the ~2.7µs ACT
            # table load off the critical path too. ----
            with tc.tile_pool(name="pp", bufs=4, space="PSUM") as pp:
                warm_sb = const.tile([128, 512], BF16, tag="warm")
                nc.vector.memset(warm_sb, 0.0)
                dume_sb = const.tile([1, 16], BF16, tag="dume")
                nc.scalar.activation(
                    out=dume_sb, in_=warm_sb[0:1, 0:16], func=EXP, scale=SCALE
                )
                warm_ps = pp.tile([128, 512], F32, tag="pp")
                for _ in range(12):
                    nc.tensor.matmul(
                        out=warm_ps, lhsT=warm_sb[:, 0:128], rhs=warm_sb[:, :],
                        start=True, stop=True,
                    )

                # ---- phase 1: qT/kT [feature, s] = w_qkv^T @ x; bias folded
                #      into the eviction (per-partition, features-major),
                #      evictions alternating ScalarE/VectorE ----
                for t in range(4):  # feature tiles: q0,q1,k0,k1
                    for nb in range(4):  # s blocks of 512
                        ps = pp.tile([128, 512], F32, tag="pp")
                        for ks in range(2):
                            nc.tensor.matmul(
                                out=ps,
                                lhsT=wqk_sb[:, ks * 512 + t * 128 : ks * 512 + t * 128 + 128],
                                rhs=xt_sb[:, ks * S + nb * 512 : ks * S + nb * 512 + 512],
                                start=(ks == 0), stop=(ks == 1),
                            )
                        dst = (qT_sb if t < 2 else kT_sb)[
                            :, (t % 2) * S + nb * 512 : (t % 2) * S + nb * 512 + 512
                        ]
                        if (t * 4 + nb) % 2 == 0:
                            nc.scalar.activation(
                                out=dst, in_=ps, func=IDN,
                                bias=bqkc_sb[:, t : t + 1], scale=1.0,
                            )
                        else:
                            nc.vector.tensor_scalar_add(
                                out=dst, in0=ps, scalar1=bqkc_sb[:, t : t + 1]
                            )

                # ---- phase 2: v (natural layout, 32-wide head slots),
                #      evictions alternating ScalarE/VectorE ----
                for st in range(16):
                    ps = pp.tile([128, 256], F32, tag="ppv")
                    for ks in range(2):
                        nc.tensor.matmul(
                            out=ps,
                            lhsT=xt_sb[:, ks * S + st * 128 : ks * S + st * 128 + 128],
                            rhs=wv_sb[:, ks * 256 : ks * 256 + 256],
                            start=(ks == 0), stop=False,
                        )
                    nc.tensor.matmul(
                        out=ps,
                        lhsT=ones1_sb[0:1, 0:128],
                        rhs=bv_sb[0:1, 0:256],
                        start=False, stop=True,
                    )
                    dst = v_sb[:, st * 256 : st * 256 + 256]
                    if st % 2 == 0:
                        nc.scalar.copy(out=dst, in_=ps)
                    else:
                        nc.vector.tensor_copy(out=dst, in_=ps)

            # ---- phase 3: attention (pp pool closed).  Runs as 8
            #      sequential passes (q-block 512) x (head-group of 4);
            #      per pass 16 key-tiles x 2 half-blocks of 2 heads.
            #      PSUM: 3 score lanes [128,1024] (6 banks; each of the
            #      two concurrent row-tiled score matmuls in a lane owns
            #      a FULL bank — two row-tiled matmuls draining into one
            #      single-port bank SRAM is a fatal PSUM collision) +
            #      1 ctx bank + 1 rowsum bank = 8. ----
            hidx = 0  # global exp-half-block counter for the engine split

            if True:
                def evict_pass(p, ctx_ps, rs_ps):
                    # engine copies out of PSUM (frees the accumulator
                    # banks), then ship the 4 rowsum rows to DRAM and
                    # gather them back as one [64, 32] tile
                    stg = nrm.tile([128, 512], F32, tag="stg", name=f"stg_{p}")
                    nc.scalar.copy(out=stg, in_=ctx_ps)
                    rss = nrm.tile([128, 512], F32, tag="rss", name=f"rss_{p}")
                    nc.vector.tensor_copy(out=rss, in_=rs_ps)
                    for hh in range(4):
                        nc.sync.dma_start(
                            out=rscr[p * 4 + hh : p * 4 + hh + 1, :],
                            in_=rss[hh * 32 : hh * 32 + 1, :],
                        )
                    rsg = nrm.tile([64, 32], F32, tag="rsg", name=f"rsg_{p}")
                    nc.sync.dma_start(
                        out=rsg,
                        in_=rscr[p * 4 : p * 4 + 4, :].rearrange(
                            "r (c k) -> (r c) k", k=32
                        ),
                    )
                    return stg, rsg

                def norm_a(p, stg, rsg):
                    # reciprocal + scatter + partition-broadcast roundtrip
                    rsgi = nrm.tile([64, 32], F32, tag="rsgi", name=f"rsgi_{p}")
                    nc.vector.reciprocal(out=rsgi, in_=rsg)
                    nc.sync.dma_start(
                        out=rscr2[p * 4 : p * 4 + 4, :].rearrange(
                            "r (c k) -> (r c) k", k=32
                        ),
                        in_=rsgi,
                    )
                    rcb = nrm.tile([128, 512], F32, tag="rcb", name=f"rcb_{p}")
                    for hh in range(4):
                        nc.sync.dma_start(
                            out=rcb[hh * 32 : hh * 32 + 32, :],
                            in_=rscr2[p * 4 + hh : p * 4 + hh + 1, :].to_broadcast((32, 512)),
                        )
                    return rcb

                def norm_b(p, stg, rcb):
                    qb, g = p // 2, p % 2
                    nc.vector.tensor_mul(
                        out=ctxT_sb[g][:, qb * 512 : qb * 512 + 512],
                        in0=stg, in1=rcb,
                    )

                with (
                    tc.tile_pool(name="scp", bufs=3, space="PSUM") as scp,
                    tc.tile_pool(name="cxp", bufs=1, space="PSUM") as cxp,
                ):
                    pending_norm = None
                    prev_rs = None
                    for p in range(8):  # pass = (q-block, head-group)
                        qb, g = p // 2, p % 2
                        ctx_ps = cxp.tile([128, 512], F32, tag="ctx", name=f"ctx_{p}")
                        rs_ps = cxp.tile([128, 512], F32, tag="rs", name=f"rs_{p}")
                        rs_started = False

                        def emit_ctx(kt, eTs):
                            # ctx + rowsum accumulation for key-tile kt —
                            # one block late so these PE matmuls overlap
                            # the next exps.  4 column-tiled matmuls per
                            # wave (distinct partition groups -> distinct
                            # PSUM write ports, safe to share the bank).
                            for hh in range(4):
                                nc.tensor.matmul(
                                    out=ctx_ps[hh * 32 : hh * 32 + 32, :],
                                    lhsT=v_sb[:, kt * 256 + (g * 4 + hh) * 32 : kt * 256 + (g * 4 + hh) * 32 + 32],
                                    rhs=eTs[hh // 2][:, (hh % 2) * 512 : (hh % 2) * 512 + 512],
                                    start=(kt == 0), stop=(kt == 15),
                                    tile_position=(0, hh * 32), skip_group_check=True,
                                )
                            for hh in range(4):
                                nc.tensor.matmul(
                                    out=rs_ps[hh * 32 : hh * 32 + 32, :],
                                    lhsT=onec_sb[:, 0:32],
                                    rhs=eTs[hh // 2][:, (hh % 2) * 512 : (hh % 2) * 512 + 512],
                                    start=(kt == 0), stop=(kt == 15),
                                    tile_position=(0, hh * 32), skip_group_check=True,
                                )

                        def filler(rs_tile, n=256):
                            # accumulate +0 into a live rowsum row: pure PE
                            # busyness, numerically a no-op
                            nc.tensor.matmul(
                                out=rs_tile[0:1, 0:n],
                                lhsT=zeroc_sb[:, 0:1],
                                rhs=warm_sb[:, 0:n],
                                start=False, stop=False, skip_group_check=True,
                            )

                        pending = None  # (kt, [eT_half0, eT_half1])
                        cur_eTs = None
                        for kt in range(16):
                            # previous pass's normalize, emitted a couple
                            # of blocks into this one so the DMA roundtrip
                            # overlaps exp work instead of head-of-line
                            # blocking the exp streams
                            if pending_norm is not None and kt == 2:
                                pp_, pstg, prsg = pending_norm
                                pending_norm = (pp_, pstg, norm_a(pp_, pstg, prsg))
                            if pending_norm is not None and kt == 4:
                                pp_, pstg, prcb = pending_norm
                                norm_b(pp_, pstg, prcb)
                                pending_norm = None
                            cur_eTs = []
                            for half in range(2):
                                eT = etp.tile([128, 1024], BF16, tag="eT")
                                sc = scp.tile([128, 1024], F32, tag="sc",
                                              name=f"sc_{p}_{kt}_{half}")
                                for j in range(2):
                                    hh = 2 * half + j
                                    nc.tensor.matmul(
                                        out=sc[:, j * 512 : j * 512 + 512],
                                        lhsT=kT_sb[32 * hh : 32 * hh + 32,
                                                   g * S + kt * 128 : g * S + kt * 128 + 128],
                                        rhs=qT_sb[32 * hh : 32 * hh + 32,
                                                  g * S + qb * 512 : g * S + qb * 512 + 512],
                                        start=True, stop=True,
                                        tile_position=(32 * hh, 0),
                                    )
                                if (hidx % 16) in DVE_SLOTS:
                                    nc.vector.tensor_scalar(
                                        out=eT.bitcast(mybir.dt.int16),
                                        in0=sc, scalar1=A16, scalar2=B16,
                                        op0=MUL, op1=ADD,
                                    )
                                else:
                                    nc.scalar.activation(
                                        out=eT, in_=sc, func=EXP, scale=SCALE,
                                    )
                                hidx += 1
                                cur_eTs.append(eT)
                            # ctx/rs wave for the previous key-tile goes
                            # AFTER both halves' scores: the scores feed
                            # the exp engines immediately, and by the time
                            # the PE reaches the ctx wave its eT dependency
                            # (previous block's half-1 exp) has had a full
                            # scores-duration to finish — no head-of-line
                            # stall
                            if pending is not None:
                                emit_ctx(*pending)
                                if pending[0] == 0:
                                    rs_started = True
                            # zero-add fillers keep the PE stream gapless
                            # so HAM never re-throttles
                            if FILLERS:
                                if rs_started:
                                    filler(rs_ps)
                                elif prev_rs is not None:
                                    filler(prev_rs)
                            pending = (kt, cur_eTs)
                        emit_ctx(*pending)
                        # eviction emitted before the next pass re-allocates
                        # these PSUM buffers
                        pending_norm = (p,) + evict_pass(p, ctx_ps, rs_ps)
                        prev_rs = rs_ps

                    # tail: normalize the last pass
                    pp_, pstg, prsg = pending_norm
                    rcb = norm_a(pp_, pstg, prsg)
                    norm_b(pp_, pstg, rcb)

            # ---- phase 4: out = ctxT^T @ w_out + b_out (K=1 ones-row
            #      matmul adds the bias) ----
            with (
                tc.tile_pool(name="op", bufs=4, space="PSUM") as op,
                tc.tile_pool(name="ev", bufs=4) as ev,
            ):
                for st in range(16):
                    ps = op.tile([128, 256], F32, tag="op")
                    nc.tensor.matmul(
                        out=ps, lhsT=ones1_sb[0:1, 0:128], rhs=bo_sb[0:1, :],
                        start=True, stop=False,
                    )
                    for g in range(2):
                        nc.tensor.matmul(
                            out=ps,
                            lhsT=ctxT_sb[g][:, st * 128 : st * 128 + 128],
                            rhs=wo_sb[:, g * 256 : g * 256 + 256],
                            start=False, stop=(g == 1),
                        )
                    ot = ev.tile([128, 256], F32, tag="ot")
                    if st % 2 == 0:
                        nc.scalar.copy(out=ot, in_=ps)
                    else:
                        nc.vector.tensor_copy(out=ot, in_=ps)
                    nc.sync.dma_start(
                        out=out[st * 128 : st * 128 + 128, :], in_=ot
                    )
    if legalize:
        _legalize_sync_waits(nc)
    return nc


_NC_CACHE = None


def _get_nc():
    global _NC_CACHE
    if _NC_CACHE is None:
        _NC_CACHE = _build_nc()
    return _NC_CACHE


def _ks_layout(a, nk, cols):
    """[nk*128, cols] -> [128, nk*cols] with [p, k*cols+c] = a[k*128+p, c]."""
    return np.ascontiguousarray(
        a.reshape(nk, 128, cols).transpose(1, 0, 2).reshape(128, nk * cols)
    )


def _prep_in_maps(x, w_qkv, b_qkv, w_out, b_out):
    x = np.asarray(x, dtype=np.float32)
    w_qkv = np.asarray(w_qkv, dtype=np.float32)
    b_qkv = np.asarray(b_qkv, dtype=np.float32)
    w_out = np.asarray(w_out, dtype=np.float32)
    b_out = np.asarray(b_out, dtype=np.float32)

    # shared (per-core identical) weight layouts
    wqk_l = _ks_layout(w_qkv[:, : 2 * H], 2, 512).astype(NPBF16)
    # v weights in natural head order (32-wide slots)
    wv_l = _ks_layout(w_qkv[:, 2 * H :], 2, 256).astype(NPBF16)
    # out projection: ctxT is packed [(head h)*32 + d] so w_out needs no
    # permutation, only the K-split layout
    wo_l = _ks_layout(w_out, 2, 256).astype(NPBF16)

    shared = {
        "wqk": wqk_l,
        "wv": wv_l,
        "bv": b_qkv[2 * H :].reshape(1, H).astype(NPBF16),
        "wo": wo_l,
        "bo": b_out.reshape(1, H).astype(NPBF16),
        "bqkc": np.ascontiguousarray(
            b_qkv[: 2 * H].astype(np.float32).reshape(4, 128).T
        ),
        "ones": np.ones((1, 512), NPBF16),
    }
    in_maps = []
    for b in range(B):
        xt = _ks_layout(np.ascontiguousarray(x[b].T), 2, S).astype(NPBF16)
        in_maps.append({"xt": xt, **shared})
    return in_maps


def kernel(x, w_qkv, b_qkv, w_out, b_out):
    in_maps = _prep_in_maps(x, w_qkv, b_qkv, w_out, b_out)
    nc = _get_nc()
    res = run_bass_kernel_spmd(nc, in_maps, list(range(N_CORES)), **TRACE_OPTS)
    global LAST_RESULT
    LAST_RESULT = res
    return np.stack([res.results[b]["out"] for b in range(B)], axis=0)


# revision 43
# speedup vs baseline: 1.5192x; 1.0282x over previous
"""Multi-head self-attention (B=8, S=2048, H=256, NH=8, HD=32) on 8 TRN2 cores.

Strategy: data-parallel over batch — each core computes full MHA for one
batch element; no collectives.

Per-core dataflow (all matmuls bf16 in / fp32 PSUM accum):
  - host ships x^T (features on partitions) so no on-device transpose
  - qkT:  q^T,k^T [feat, s] = w_qkv^T @ x — feature-major so each head's
    32 q/k features land on one 32-partition strip; bias folded into the
    eviction (split between ScalarE and VectorE, both idle here)
  - the attention loop runs over 256 blocks: (q-block 256) x (key-tile
    128) x (head-group of 4).  Per block, ONE [128, 4*256] scores PSUM
    lane is filled by 4 row-tiled PE matmuls (tile_position=(32i,0),
    K=32), consumed by ONE 1024-column exp op.  THREE lanes rotate over
    blocks so consecutive blocks' exps never wait on each other's
    buffer WAR — both exp engines always have a queued op.
  - softmax exp is the kernel bottleneck (NH*S*S = 33.5M elements/core,
    and exp natively runs only on ScalarE at 1 col/cycle).  The exp work
    is therefore SPLIT between two engines:
      * ScalarE share: ACTIVATE(Exp, scale=1/sqrt(HD)) from PSUM
      * VectorE share: one TENSOR_SCALAR computing the Schraudolph bit
        trick — i16 = trunc(score * (128*log2(e)/sqrt(HD)) + (127*128-C))
        written through an int16 bitcast of the bf16 eT tile.  The int16
        bit pattern IS bf16(exp(score/sqrt(HD))) up to ~2% sawtooth error
        which largely cancels under softmax renormalization (validated:
        global rel err 0.008 vs 0.005 for exact exp).
    A 16-block pattern interleaves the engines ~9:7 to balance
    ScalarE@1.2GHz against VectorE@0.96GHz + its other work.
  - softmax max-subtraction is skipped (scores are O(1), fp32 PSUM)
  - ctx^T accumulated over key tiles with 4x column-tiled PE
    (tile_position=(0,32h)): all 8 heads of BOTH groups land fully
    packed in ONE [128,512] PSUM bank (g0 cols 0:256, g1 cols 256:512);
    rowsums likewise accumulate as M=1 column-tiled matmuls against a
    ones column into a second shared bank (row h*32, same col split)
  - dep-free filler matmuls (zeros column accumulating +0 into the live
    rowsum rows) keep the PE instruction stream gapless so the HAM
    activity monitor never re-throttles the PE clock 2.4 -> 1.2 GHz
    (without them a MID window fires ~20µs in and the whole attention
    loop runs at half clock); the tail of the v-projection is deferred
    into the first q-block for the same reason
  - normalization off the critical path: ctx/rowsum banks evicted by
    single full-tile engine copies into [128,512] fp32 staging (frees
    the accumulators), rowsum rows shipped via DRAM into one [128,16]
    tile, one VectorE reciprocal, scattered back and partition-broadcast
    via DRAM, then 2 [128,256] multiplies write the normalized bf16
    ctxT.  All pieces are emitted a few blocks into the NEXT q-block so
    the DMA round-trips overlap exp work instead of head-of-line
    blocking the exp streams.
  - out = ctxT^T @ w_out + b_out: ctxT is fully packed (2 tiles of
    4 heads x 32 rows), w_out needs no permutation, bias comes from a
    K=1 ones-row matmul.
"""
import math

import numpy as np
import ml_dtypes

import bass_rust
import concourse.bass as bass
import concourse.mybir as mybir
import concourse.tile as tile
from concourse.vector_clock import ScopedClock
from concourse.bass_utils import run_bass_kernel_spmd

BF16 = mybir.dt.bfloat16
F32 = mybir.dt.float32
NPBF16 = ml_dtypes.bfloat16

B, S, H = 8, 2048, 256
NH, HD = 8, 32
SCALE = 1.0 / float(np.sqrt(HD))
N_CORES = 8
QB = 512  # q-block size
NQB = S // QB  # 4 q-blocks (x2 head-group passes = 8 passes)

# Schraudolph-exp constants for the VectorE share: bf16 bit pattern of
# exp(SCALE*x) ~= trunc(x * A16 + B16) interpreted as int16.
A16 = SCALE * 128.0 / math.log(2.0)
B16 = 127.0 * 128.0 - 6.0

# Which of each 16 consecutive (qb,kt,g) exp blocks go to VectorE
# (7 of 16; the rest go to ScalarE).  Evenly interleaved.
DVE_SLOTS = frozenset(i for i in range(16) if (i * 7) % 16 < 7)

FILLERS = True  # warm-keeper filler matmuls (see phase 3)

# Set by a test harness to collect HW timing: {"trace": bool, "trace_cores": [...]}
TRACE_OPTS = {}
LAST_RESULT = None

def _legalize_sync_waits(nc):
    """The walrus build here rejects >1 sync wait per instruction, but Tile
    freely emits 2-3 (and the exit drain up to ~27).  Move excess waits onto
    same-engine NoOp carriers inserted immediately before the offending
    instruction — identical semantics (the engine blocks on each wait in
    program order)."""
    n = 0
    for f in nc.m.functions:
        for bb in f.blocks:
            insts = bb.instructions  # live list
            i = 0
            while i < len(insts):
                inst = insts[i]
                si = inst.sync_info
                if si is not None and len(si.on_wait) > 1:
                    waits = list(si.on_wait)
                    carriers = []
                    for w in waits[:-1]:
                        carriers.append(
                            mybir.InstNoOp(
                                name=f"{inst.name}-w{n}",
                                sync_info=mybir.SyncInfo(on_wait=[w], on_update=[]),
                                bass_nofuse=True,
                                engine=inst.engine,
                            )
                        )
                        n += 1
                    inst.sync_info = bass_rust.SyncInfo(
                        on_wait=waits[-1:], on_update=list(si.on_update)
                    )
                    insts[i:i] = carriers
                    i += len(carriers)
                i += 1
    return n


def _build_nc(legalize=True):
    nc = bass.Bass()
    xt = nc.dram_tensor("xt", [128, 2 * S], BF16, kind="ExternalInput")
    wqk = nc.dram_tensor("wqk", [128, 2 * 512], BF16, kind="ExternalInput")
    bv = nc.dram_tensor("bv", [1, 256], BF16, kind="ExternalInput")
    wv = nc.dram_tensor("wv", [128, 2 * 256], BF16, kind="ExternalInput")
    wo = nc.dram_tensor("wo", [128, 2 * 256], BF16, kind="ExternalInput")
    bo = nc.dram_tensor("bo", [1, 256], BF16, kind="ExternalInput")
    bqkc = nc.dram_tensor("bqkc", [128, 4], F32, kind="ExternalInput")
    ones = nc.dram_tensor("ones", [1, 512], BF16, kind="ExternalInput")
    out = nc.dram_tensor("out", [S, H], F32, kind="ExternalOutput")
    # scratch for the rowsum gather / reciprocal-broadcast DMA roundtrips
    # (SBUF APs cannot have a zero partition step, DRAM APs can)
    rscr = nc.dram_tensor("rscr", [32, 512], F32)
    rscr2 = nc.dram_tensor("rscr2", [32, 512], F32)

    EXP = mybir.ActivationFunctionType.Exp
    IDN = mybir.ActivationFunctionType.Identity
    MUL = mybir.AluOpType.mult
    ADD = mybir.AluOpType.add

    with tile.TileContext(nc) as tc:
        with (
            tc.tile_pool(name="const", bufs=1) as const,
            tc.tile_pool(name="etp", bufs=6) as etp,
            tc.tile_pool(name="nrm", bufs=2) as nrm,
        ):
            xt_sb = const.tile([128, 2 * S], BF16, tag="xt")
            nc.sync.dma_start(out=xt_sb, in_=xt[:, :])
            wqk_sb = const.tile([128, 2 * 512], BF16, tag="wqk")
            nc.sync.dma_start(out=wqk_sb, in_=wqk[:, :])
            wv_sb = const.tile([128, 2 * 256], BF16, tag="wv")
            nc.sync.dma_start(out=wv_sb, in_=wv[:, :])
            wo_sb = const.tile([128, 2 * 256], BF16, tag="wo")
            nc.sync.dma_start(out=wo_sb, in_=wo[:, :])
            bv_sb = const.tile([1, 256], BF16, tag="bv")
            nc.sync.dma_start(out=bv_sb, in_=bv[:, :])
            bo_sb = const.tile([1, 256], BF16, tag="bo")
            nc.sync.dma_start(out=bo_sb, in_=bo[:, :])
            ones1_sb = const.tile([1, 128], BF16, tag="ones1")
            nc.sync.dma_start(out=ones1_sb, in_=ones[0:1, 0:128])
            bqkc_sb = const.tile([128, 4], F32, tag="bqkc")
            nc.sync.dma_start(out=bqkc_sb, in_=bqkc[:, :])
            # ones block for the rowsum matmuls (K=128, M=32: 32 identical
            # rowsum rows cost the same PE time as 1 and leave no stale
            # bytes in the shared PSUM bank) and a zeros column for the
            # harmless warm-keeper fillers
            onec_sb = const.tile([128, 32], BF16, tag="onec")
            nc.vector.memset(onec_sb, 1.0)
            zeroc_sb = const.tile([128, 1], BF16, tag="zeroc")
            nc.vector.memset(zeroc_sb, 0.0)

            qT_sb = const.tile([128, 2 * S], BF16, tag="qT")
            kT_sb = const.tile([128, 2 * S], BF16, tag="kT")
            v_sb = const.tile([128, 16 * 256], BF16, tag="v")
            # fully-packed normalized ctx^T: tile g holds heads 4g..4g+3,
            # rows h*32..h*32+32 = head (4g+h) features, cols = q positions
            ctxT_sb = [
                const.tile([128, S], BF16, tag=f"ctxT{g}", name=f"ctxT{g}")
                for g in range(2)
            ]

            # ---- phase 0: HAM warmup — ~6µs of dep-free back-to-back
            # matmuls so the PE clock gate opens (1.2 -> 2.4 GHz) before the
            # real work; garbage values land in a scratch PSUM bank that is
            # never read.  A dummy exp on ScalarE pulls the ~2.7µs ACT
            # table load off the critical path too. ----
            with tc.tile_pool(name="pp", bufs=4, space="PSUM") as pp:
                warm_sb = const.tile([128, 512], BF16, tag="warm")
                nc.vector.memset(warm_sb, 0.0)
                dume_sb = const.tile([1, 16], BF16, tag="dume")
                nc.scalar.activation(
                    out=dume_sb, in_=warm_sb[0:1, 0:16], func=EXP, scale=SCALE
                )
                warm_ps = pp.tile([128, 512], F32, tag="pp")
                for _ in range(12):
                    nc.tensor.matmul(
                        out=warm_ps, lhsT=warm_sb[:, 0:128], rhs=warm_sb[:, :],
                        start=True, stop=True,
                    )

                # ---- phase 1: qT/kT [feature, s] = w_qkv^T @ x; bias folded
                #      into the eviction (per-partition, features-major),
                #      evictions alternating ScalarE/VectorE ----
                for t in range(4):  # feature tiles: q0,q1,k0,k1
                    for nb in range(4):  # s blocks of 512
                        ps = pp.tile([128, 512], F32, tag="pp")
                        for ks in range(2):
                            nc.tensor.matmul(
                                out=ps,
                                lhsT=wqk_sb[:, ks * 512 + t * 128 : ks * 512 + t * 128 + 128],
                                rhs=xt_sb[:, ks * S + nb * 512 : ks * S + nb * 512 + 512],
                                start=(ks == 0), stop=(ks == 1),
                            )
                        dst = (qT_sb if t < 2 else kT_sb)[
                            :, (t % 2) * S + nb * 512 : (t % 2) * S + nb * 512 + 512
                        ]
                        if (t * 4 + nb) % 2 == 0:
                            nc.scalar.activation(
                                out=dst, in_=ps, func=IDN,
                                bias=bqkc_sb[:, t : t + 1], scale=1.0,
                            )
                        else:
                            nc.vector.tensor_scalar_add(
                                out=dst, in0=ps, scalar1=bqkc_sb[:, t : t + 1]
                            )

                # ---- phase 2: v (natural layout, 32-wide head slots),
                #      evictions alternating ScalarE/VectorE ----
                for st in range(16):
                    ps = pp.tile([128, 256], F32, tag="ppv")
                    for ks in range(2):
                        nc.tensor.matmul(
                            out=ps,
                            lhsT=xt_sb[:, ks * S + st * 128 : ks * S + st * 128 + 128],
                            rhs=wv_sb[:, ks * 256 : ks * 256 + 256],
                            start=(ks == 0), stop=False,
                        )
                    nc.tensor.matmul(
                        out=ps,
                        lhsT=ones1_sb[0:1, 0:128],
                        rhs=bv_sb[0:1, 0:256],
                        start=False, stop=True,
                    )
                    dst = v_sb[:, st * 256 : st * 256 + 256]
                    if st % 2 == 0:
                        nc.scalar.copy(out=dst, in_=ps)
                    else:
                        nc.vector.tensor_copy(out=dst, in_=ps)

            # ---- phase 3: attention (pp pool closed).  Runs as 8
            #      sequential passes (q-block 512) x (head-group of 4);
            #      per pass 16 key-tiles x 2 half-blocks of 2 heads.
            #      PSUM: 3 score lanes [128,1024] (6 banks; each of the
            #      two concurrent row-tiled score matmuls in a lane owns
            #      a FULL bank — two row-tiled matmuls draining into one
            #      single-port bank SRAM is a fatal PSUM collision) +
            #      1 ctx bank + 1 rowsum bank = 8. ----
            hidx = 0  # global exp-half-block counter for the engine split

            if True:
                def evict_pass(p, ctx_ps, rs_ps):
                    # engine copies out of PSUM (frees the accumulator
                    # banks), then ship the 4 rowsum rows to DRAM and
                    # gather them back as one [64, 32] tile
                    stg = nrm.tile([128, 512], F32, tag="stg", name=f"stg_{p}")
                    nc.scalar.copy(out=stg, in_=ctx_ps)
                    rss = nrm.tile([128, 512], F32, tag="rss", name=f"rss_{p}")
                    nc.vector.tensor_copy(out=rss, in_=rs_ps)
                    for hh in range(4):
                        nc.sync.dma_start(
                            out=rscr[p * 4 + hh : p * 4 + hh + 1, :],
                            in_=rss[hh * 32 : hh * 32 + 1, :],
                        )
                    rsg = nrm.tile([64, 32], F32, tag="rsg", name=f"rsg_{p}")
                    nc.sync.dma_start(
                        out=rsg,
                        in_=rscr[p * 4 : p * 4 + 4, :].rearrange(
                            "r (c k) -> (r c) k", k=32
                        ),
                    )
                    return stg, rsg

                def norm_a(p, stg, rsg):
                    # reciprocal + scatter + partition-broadcast roundtrip
                    rsgi = nrm.tile([64, 32], F32, tag="rsgi", name=f"rsgi_{p}")
                    nc.vector.reciprocal(out=rsgi, in_=rsg)
                    nc.sync.dma_start(
                        out=rscr2[p * 4 : p * 4 + 4, :].rearrange(
                            "r (c k) -> (r c) k", k=32
                        ),
                        in_=rsgi,
                    )
                    rcb = nrm.tile([128, 512], F32, tag="rcb", name=f"rcb_{p}")
                    for hh in range(4):
                        nc.sync.dma_start(
                            out=rcb[hh * 32 : hh * 32 + 32, :],
                            in_=rscr2[p * 4 + hh : p * 4 + hh + 1, :].to_broadcast((32, 512)),
                        )
                    return rcb

                def norm_b(p, stg, rcb):
                    qb, g = p // 2, p % 2
                    nc.vector.tensor_mul(
                        out=ctxT_sb[g][:, qb * 512 : qb * 512 + 512],
                        in0=stg, in1=rcb,
                    )

                with (
                    tc.tile_pool(name="scp", bufs=3, space="PSUM") as scp,
                    tc.tile_pool(name="cxp", bufs=1, space="PSUM") as cxp,
                ):
                    pending_norm = None
                    prev_rs = None
                    for p in range(8):  # pass = (q-block, head-group)
                        qb, g = p // 2, p % 2
                        ctx_ps = cxp.tile([128, 512], F32, tag="ctx", name=f"ctx_{p}")
                        rs_ps = cxp.tile([128, 512], F32, tag="rs", name=f"rs_{p}")
                        rs_started = False

                        def emit_ctx(kt, eTs):
                            # ctx + rowsum accumulation for key-tile kt —
                            # one block late so these PE matmuls overlap
                            # the next exps.  4 column-tiled matmuls per
                            # wave (distinct partition groups -> distinct
                            # PSUM write ports, safe to share the bank).
                            for hh in range(4):
                                nc.tensor.matmul(
                                    out=ctx_ps[hh * 32 : hh * 32 + 32, :],
                                    lhsT=v_sb[:, kt * 256 + (g * 4 + hh) * 32 : kt * 256 + (g * 4 + hh) * 32 + 32],
                                    rhs=eTs[hh // 2][:, (hh % 2) * 512 : (hh % 2) * 512 + 512],
                                    start=(kt == 0), stop=(kt == 15),
                                    tile_position=(0, hh * 32), skip_group_check=True,
                                )
                            for hh in range(4):
                                nc.tensor.matmul(
                                    out=rs_ps[hh * 32 : hh * 32 + 32, :],
                                    lhsT=onec_sb[:, 0:32],
                                    rhs=eTs[hh // 2][:, (hh % 2) * 512 : (hh % 2) * 512 + 512],
                                    start=(kt == 0), stop=(kt == 15),
                                    tile_position=(0, hh * 32), skip_group_check=True,
                                )

                        def filler(rs_tile, n=256):
                            # accumulate +0 into a live rowsum row: pure PE
                            # busyness, numerically a no-op
                            nc.tensor.matmul(
                                out=rs_tile[0:1, 0:n],
                                lhsT=zeroc_sb[:, 0:1],
                                rhs=warm_sb[:, 0:n],
                                start=False, stop=False, skip_group_check=True,
                            )

                        pendings = []  # [(kt, [eT_half0, eT_half1]), ...]
                        cur_eTs = None
                        for kt in range(16):
                            # previous pass's normalize, emitted a couple
                            # of blocks into this one so the DMA roundtrip
                            # overlaps exp work instead of head-of-line
                            # blocking the exp streams
                            if pending_norm is not None and kt == 2:
                                pp_, pstg, prsg = pending_norm
                                pending_norm = (pp_, pstg, norm_a(pp_, pstg, prsg))
                            if pending_norm is not None and kt == 4:
                                pp_, pstg, prcb = pending_norm
                                norm_b(pp_, pstg, prcb)
                                pending_norm = None
                            cur_eTs = []
                            for half in range(2):
                                eT = etp.tile([128, 1024], BF16, tag="eT")
                                sc = scp.tile([128, 1024], F32, tag="sc",
                                              name=f"sc_{p}_{kt}_{half}")
                                for j in range(2):
                                    hh = 2 * half + j
                                    nc.tensor.matmul(
                                        out=sc[:, j * 512 : j * 512 + 512],
                                        lhsT=kT_sb[32 * hh : 32 * hh + 32,
                                                   g * S + kt * 128 : g * S + kt * 128 + 128],
                                        rhs=qT_sb[32 * hh : 32 * hh + 32,
                                                  g * S + qb * 512 : g * S + qb * 512 + 512],
                                        start=True, stop=True,
                                        tile_position=(32 * hh, 0),
                                    )
                                if (hidx % 16) in DVE_SLOTS:
                                    nc.vector.tensor_scalar(
                                        out=eT.bitcast(mybir.dt.int16),
                                        in0=sc, scalar1=A16, scalar2=B16,
                                        op0=MUL, op1=ADD,
                                    )
                                else:
                                    nc.scalar.activation(
                                        out=eT, in_=sc, func=EXP, scale=SCALE,
                                    )
                                hidx += 1
                                cur_eTs.append(eT)
                            # ctx/rs wave emitted TWO key-tiles late and
                            # after the scores: its eT dependencies are
                            # long resolved, so the PE streams wave after
                            # wave without dependency restarts
                            pendings.append((kt, cur_eTs))
                            if len(pendings) > 2:
                                pend = pendings.pop(0)
                                emit_ctx(*pend)
                                if pend[0] == 0:
                                    rs_started = True
                            # zero-add fillers keep the PE stream gapless
                            # so HAM never re-throttles
                            if FILLERS:
                                if rs_started:
                                    filler(rs_ps)
                                elif prev_rs is not None:
                                    filler(prev_rs)
                        for pend in pendings:
                            emit_ctx(*pend)
                        # eviction emitted before the next pass re-allocates
                        # these PSUM buffers
                        pending_norm = (p,) + evict_pass(p, ctx_ps, rs_ps)
                        prev_rs = rs_ps

                    # tail: normalize the last pass
                    pp_, pstg, prsg = pending_norm
                    rcb = norm_a(pp_, pstg, prsg)
                    norm_b(pp_, pstg, rcb)

            # ---- phase 4: out = ctxT^T @ w_out + b_out (K=1 ones-row
            #      matmul adds the bias) ----
            with (
                tc.tile_pool(name="op", bufs=4, space="PSUM") as op,
                tc.tile_pool(name="ev", bufs=4) as ev,
            ):
                for st in range(16):
                    ps = op.tile([128, 256], F32, tag="op")
                    nc.tensor.matmul(
                        out=ps, lhsT=ones1_sb[0:1, 0:128], rhs=bo_sb[0:1, :],
                        start=True, stop=False,
                    )
                    for g in range(2):
                        nc.tensor.matmul(
                            out=ps,
                            lhsT=ctxT_sb[g][:, st * 128 : st * 128 + 128],
                            rhs=wo_sb[:, g * 256 : g * 256 + 256],
                            start=False, stop=(g == 1),
                        )
                    ot = ev.tile([128, 256], F32, tag="ot")
                    if st % 2 == 0:
                        nc.scalar.copy(out=ot, in_=ps)
                    else:
                        nc.vector.tensor_copy(out=ot, in_=ps)
                    nc.sync.dma_start(
                        out=out[st * 128 : st * 128 + 128, :], in_=ot
                    )
    if legalize:
        _legalize_sync_waits(nc)
    return nc


_NC_CACHE = None


def _get_nc():
    global _NC_CACHE
    if _NC_CACHE is None:
        _NC_CACHE = _build_nc()
    return _NC_CACHE


def _ks_layout(a, nk, cols):
    """[nk*128, cols] -> [128, nk*cols] with [p, k*cols+c] = a[k*128+p, c]."""
    return np.ascontiguousarray(
        a.reshape(nk, 128, cols).transpose(1, 0, 2).reshape(128, nk * cols)
    )


def _prep_in_maps(x, w_qkv, b_qkv, w_out, b_out):
    x = np.asarray(x, dtype=np.float32)
    w_qkv = np.asarray(w_qkv, dtype=np.float32)
    b_qkv = np.asarray(b_qkv, dtype=np.float32)
    w_out = np.asarray(w_out, dtype=np.float32)
    b_out = np.asarray(b_out, dtype=np.float32)

    # shared (per-core identical) weight layouts
    wqk_l = _ks_layout(w_qkv[:, : 2 * H], 2, 512).astype(NPBF16)
    # v weights in natural head order (32-wide slots)
    wv_l = _ks_layout(w_qkv[:, 2 * H :], 2, 256).astype(NPBF16)
    # out projection: ctxT is packed [(head h)*32 + d] so w_out needs no
    # permutation, only the K-split layout
    wo_l = _ks_layout(w_out, 2, 256).astype(NPBF16)

    shared = {
        "wqk": wqk_l,
        "wv": wv_l,
        "bv": b_qkv[2 * H :].reshape(1, H).astype(NPBF16),
        "wo": wo_l,
        "bo": b_out.reshape(1, H).astype(NPBF16),
        "bqkc": np.ascontiguousarray(
            b_qkv[: 2 * H].astype(np.float32).reshape(4, 128).T
        ),
        "ones": np.ones((1, 512), NPBF16),
    }
    in_maps = []
    for b in range(B):
        xt = _ks_layout(np.ascontiguousarray(x[b].T), 2, S).astype(NPBF16)
        in_maps.append({"xt": xt, **shared})
    return in_maps


def kernel(x, w_qkv, b_qkv, w_out, b_out):
    in_maps = _prep_in_maps(x, w_qkv, b_qkv, w_out, b_out)
    nc = _get_nc()
    res = run_bass_kernel_spmd(nc, in_maps, list(range(N_CORES)), **TRACE_OPTS)
    global LAST_RESULT
    LAST_RESULT = res
    return np.stack([res.results[b]["out"] for b in range(B)], axis=0)
